# revision 36
# baseline (speedup 1.0000x reference)
"""HSMNet cost-volume + disparity softmax-regression on 8 Trainium2 NeuronCores.

Reference computation (per batch b):
  cost[c,d,h,w] = |ref[c,h,w] - tgt[c,h,w-d]| for w>=d else 0
  cost_agg[d,h,w] = sum_c cost
  pred[h,w] = sum_d d * softmax_d(cost_agg)

Key identity: |a-b| = 2*max(a,b) - a - b, so
  cost_agg[d,p] = 2*sum_c max(ref[c,p], tgt[c,p-d]) - R[p] - T[p-d]
with R = sum_c ref, T = sum_c tgt. R[p] is constant over d and cancels in the
softmax; the logits used are G[d,p] = 2*S_d[p] - T[p-d]. This removes the
entire elementwise-abs pass; the elementwise work is one tensor_tensor max per
disparity, and several disparities are packed per DVE instruction via an
overlapping-window access pattern (window stride 1 over the pack dim).

Sharding: 8 cores = 4 batches x 2 h-halves (40 rows x 160 cols = 6400 px).
Layout: pixels packed as 4 quarter-groups of 1600 on partitions (c + 32g);
tgt has a 23-col halo so tgt[c, p-d] is a pure column offset.

Per core:
  - 5 batched input DMAs (f32) on the sync queue; casts f32->f16 on GPSIMD
  - DVE: packed tensor_tensor max ops [128, k*800] f16
  - PE: S_d reduced over c by matmuls accumulating into PSUM [96, 2048]
    (row 4j+g, j=23-d, i.e. quadrant q=j//8 + sliding one-hot weights);
    T = sum_c tgt via the same weight tile; T/2 replicated into T_full
    [96,1600] via a DRAM bounce (2 DMAs, 3-dim strided read); -I matmuls
    subtract T_full and the 5000*invalid mask so exp zeroes invalid entries.
  - ACT Exp (scale=2) evacuates PSUM -> E [96,1600] bf16 per 400-col chunk;
    chunks 0-1 pipeline inside the half-2 DVE stream.
  - PE: lnd weights contract E -> den/num [8, 2048] PSUM; DVE evacuates.
  - host: pred = num/den.
"""
import os
import sys
import threading

for _p in ("/opt/trn_rl_repo",):
    if os.path.isdir(_p) and _p not in sys.path:
        sys.path.insert(0, _p)

import numpy as np
import ml_dtypes

import concourse.bacc as bacc
import concourse.mybir as mybir
from concourse.tile import TileContext
from concourse.bass_utils import run_bass_kernel_spmd

dt = mybir.dt

# problem shape (hardcoded per spec)
B, C, H, W = 4, 32, 80, 160
D = 24
HP = H // 2            # rows per core
PIX = HP * W           # 6400 pixels per core
QW = PIX // 4          # 1600 per quarter-group
HW_ = QW // 2          # 800 per half
PAD = 23               # halo columns in front of tgt
N_CORES = 8
MASK_BIAS = 5000.0     # pre-2x logit bias at invalid entries; exp -> 0

# d-pack sizes per 400-col chunk (sum 24); chunk 0 starts small (PE warmup)
PACKS0 = tuple(int(x) for x in os.environ.get("HSM_PACKS0", "2,8,8,6").split(","))
PACKSN = tuple(int(x) for x in os.environ.get("HSM_PACKSN", "8,8,8").split(","))
assert sum(PACKS0) == D and sum(PACKSN) == D
S_BUFS = int(os.environ.get("HSM_S_BUFS", "5"))
CW = 400               # pipeline chunk width (== PSUM bank chunk)


def _win(ap, k):
    """Overlapping-window view: [128, n] slice -> [128, k, n] where window
    j reads columns shifted by +j (stride-1 over the pack dim)."""
    v = ap.unsqueeze(1)
    v.ap[1] = [1, k]
    return v


def _rep(ap, k):
    """Repeat view: [128, n] slice -> [128, k, n], same columns per window."""
    v = ap.unsqueeze(1)
    v.ap[1] = [0, k]
    return v


def _build_program():
    nc = bacc.Bacc("TRN2", target_bir_lowering=False)
    ref_h = nc.dram_tensor("ref", [C, PIX], dt.float32, kind="ExternalInput")
    tgt_h = nc.dram_tensor("tgt", [C, PIX], dt.float32, kind="ExternalInput")
    # consts blob: cols 0:60 wS (f16), 60:156 wNI (bf16), 156:164 lnd (bf16)
    consts_h = nc.dram_tensor("consts", [128, 164], dt.uint16,
                              kind="ExternalInput")
    maskcT_h = nc.dram_tensor("maskcT", [96, QW], dt.bfloat16,
                              kind="ExternalInput")
    # DRAM bounce buffer for the T replication (ExternalOutput so the
    # runtime binds a real buffer; host ignores it)
    dramT = nc.dram_tensor("Tdbg", [4, PAD + QW], dt.bfloat16,
                           kind="ExternalOutput")
    out_h = nc.dram_tensor("out", [8, QW], dt.float32, kind="ExternalOutput")

    Alu = mybir.AluOpType
    Act = mybir.ActivationFunctionType

    # packed [128, x] views of the inputs: partition (g c) <-> hbm (c, 1600g+x)
    def _packedv(th, x0, x1):
        v = th[:, x0:x1].copy()
        v.ap[0] = [QW, 4]
        v.ap.insert(1, [PIX, C])
        return v

    with TileContext(nc) as tc:
        with tc.tile_pool(name="const", bufs=1) as cpool, \
             tc.tile_pool(name="work", bufs=1) as wpool, \
             tc.tile_pool(name="spool", bufs=S_BUFS) as spool:
            consts_sb = cpool.tile([128, 164], dt.uint16)
            wS = consts_sb[:, 0:60].bitcast(dt.float16)
            wNI = consts_sb[0:96, 60:156].bitcast(dt.bfloat16)
            lnd = consts_sb[0:96, 156:164].bitcast(dt.bfloat16)

            refS = [wpool.tile([128, CW], dt.float32, name=f"refS{i}")
                    for i in range(4)]
            tgtS = [wpool.tile([128, PAD + CW], dt.float32, name=f"tgtS{i}")
                    for i in range(4)]
            ref16 = [wpool.tile([128, CW], dt.float16, name=f"r16{i}")
                     for i in range(4)]
            tgt16h = wpool.tile([128, PAD + QW], dt.float16)
            T_hs = wpool.tile([4, PAD + QW], dt.bfloat16)   # T/2 with halo
            T_full = wpool.tile([96, QW], dt.bfloat16)
            maskcT_sb = wpool.tile([96, QW], dt.bfloat16)
            E = [wpool.tile([96, CW], dt.bfloat16, name=f"E{i}")
                 for i in range(4)]
            out_sb = [wpool.tile([8, HW_], dt.float32, name=f"o{i}")
                      for i in range(2)]

            # g=0 halo cols (before pixel 0) are zero
            nc.vector.memset(tgtS[0][0:32, 0:PAD], 0.0)

            # ---- input loads (sync queue), chunk-pipelined ----
            nc.sync.dma_start(consts_sb[:], consts_h[:])
            nc.sync.dma_start(tgtS[0][:, PAD:PAD + CW], _packedv(tgt_h, 0, CW))
            # halo for g>=1: global cols 1600g-23..1600g
            halo_in = tgt_h[:, QW - PAD:QW].copy()
            halo_in.ap[0] = [QW, 3]
            halo_in.ap.insert(1, [PIX, C])
            nc.sync.dma_start(tgtS[0][32:128, 0:PAD], halo_in)
            nc.sync.dma_start(refS[0][:], _packedv(ref_h, 0, CW))
            nc.sync.dma_start(maskcT_sb[:], maskcT_h[:])
            for cc in range(1, 4):
                nc.sync.dma_start(tgtS[cc][:],
                                  _packedv(tgt_h, CW * cc - PAD, CW * cc + CW))
                nc.sync.dma_start(refS[cc][:], _packedv(ref_h, CW * cc, CW * (cc + 1)))

            with tc.tile_pool(name="cost", bufs=1, space="PSUM") as qpool, \
                 tc.tile_pool(name="nd", bufs=1, space="PSUM") as npool:
                cost = [qpool.tile([96, 512], dt.float32, name=f"cost{i}")
                        for i in range(4)]
                nd = npool.tile([8, 2048], dt.float32)
                wT = wS[:, 28:32]  # plain c+32g -> g ones reduction

                started = set()  # (q, cc) PSUM regions already initialized

                def emit_pack(d0, k, cc):
                    # one DVE max op for disparities d0..d0+k-1 (descending
                    # window order), then k channel-reduce matmuls.
                    c0 = CW * cc
                    dhi = d0 + k - 1
                    s = spool.tile([128, k * CW], dt.float16, tag="s",
                                   name=f"s_{cc}_{d0}")
                    base = PAD - dhi + c0
                    nc.vector.tensor_tensor(
                        s[:].rearrange("p (k x) -> p k x", x=CW),
                        _rep(ref16[cc][:], k),
                        _win(tgt16h[:, base:base + CW], k),
                        Alu.max)
                    # window jw of in1 starts at base+jw -> shift PAD-d with
                    # d = dhi-jw (in0 is a broadcast repeat of ref)
                    for jw in range(k):
                        d = dhi - jw
                        j = (D - 1) - d
                        u, q = j % 8, j // 8
                        first = (q, cc) not in started
                        started.add((q, cc))
                        nc.tensor.matmul(
                            cost[cc][32 * q:32 * q + 32, 0:CW],
                            wS[:, 28 - 4 * u:60 - 4 * u],
                            s[:, jw * CW:jw * CW + CW],
                            start=first, stop=False,
                            skip_group_check=True)

                def emit_corr(cc):
                    # T/2 + mask corrections close chunk cc, then exp
                    nc.tensor.matmul(cost[cc][0:96, 0:CW],
                                     wNI, maskcT_sb[:, CW * cc:CW * cc + CW],
                                     start=False, stop=False,
                                     skip_group_check=True)
                    nc.tensor.matmul(cost[cc][0:96, 0:CW],
                                     wNI, T_full[:, CW * cc:CW * cc + CW],
                                     start=False, stop=True,
                                     skip_group_check=True)
                    nc.scalar.activation(E[cc][:], cost[cc][0:96, 0:CW],
                                         Act.Exp, scale=2.0)

                def emit_nd(cc):
                    nc.tensor.matmul(nd[0:8, 512 * cc:512 * cc + CW], lnd,
                                     E[cc][:], start=True, stop=True)

                def emit_evac(i, eng):
                    srcv = nd[0:8, 1024 * i:1024 * i + 1024]
                    srcv = srcv.rearrange("p (k x) -> p k x", k=2)[:, :, 0:CW]
                    dst = out_sb[i][:].rearrange("p (k x) -> p k x", x=CW)
                    eng(dst, srcv)

                # T-red source ranges per chunk and 512-aligned sub-splits
                tred = {0: [(0, 423)], 1: [(423, 512), (512, 823)],
                        2: [(823, 1024), (1024, 1223)],
                        3: [(1223, 1536), (1536, 1623)]}

                def emit_casts(cc):
                    d0c = 0 if cc == 0 else PAD
                    nc.scalar.copy(tgt16h[:, CW * cc + d0c:CW * cc + PAD + CW],
                                   tgtS[cc][:, d0c:PAD + CW])
                    nc.vector.tensor_copy(ref16[cc][:], refS[cc][:])

                # T-red source ranges per chunk and 512-aligned sub-splits
                tred = {0: [(0, 423)], 1: [(423, 512), (512, 823)],
                        2: [(823, 1024), (1024, 1223)],
                        3: [(1223, 1536), (1536, 1623)]}

                emit_casts(0)
                for cc in range(4):
                    # T-reduce chunk (PE) into nd temp, halve to bf16 (ACT),
                    # bounce via DRAM (sync queue is idle after the loads)
                    lo, hi = tred[cc][0][0], tred[cc][-1][1]
                    for a, b in tred[cc]:
                        nc.tensor.matmul(nd[0:4, a:b], wT, tgt16h[:, a:b],
                                         start=True, stop=True)
                    nc.scalar.mul(T_hs[:, lo:hi], nd[0:4, lo:hi], 0.5)
                    nc.sync.dma_start(dramT[:, lo:hi], T_hs[:, lo:hi])
                    tin = dramT[:, CW * cc:CW * cc + CW].copy()
                    tin.ap[0] = [1, D]            # j
                    tin.ap[1] = [PAD + QW, 4]     # g
                    tin.ap.append([1, CW])        # x ; elem = g*1623+j+x+off
                    nc.sync.dma_start(T_full[:, CW * cc:CW * (cc + 1)], tin)
                    # the d-packs + channel reduction for this chunk; next
                    # chunk's casts slot in after the first pack so they
                    # precede this chunk's exp in the ACT/DVE queues
                    packs, d0 = [], 0
                    for k in (PACKS0 if cc == 0 else PACKSN):
                        packs.append((d0, k)); d0 += k
                    emit_pack(*packs[0], cc)
                    if cc < 3:
                        emit_casts(cc + 1)
                    for p in packs[1:]:
                        emit_pack(*p, cc)
                    emit_corr(cc)
                    if cc >= 1:
                        emit_nd(cc - 1)
                    if cc == 3:
                        emit_evac(0, nc.scalar.copy)
                        nc.sync.dma_start(out_h[:, 0:HW_], out_sb[0][:])
                emit_nd(3)
                emit_evac(1, nc.vector.tensor_copy)
                nc.sync.dma_start(out_h[:, HW_:QW], out_sb[1][:])

    nc.compile()
    return nc


def _host_constants():
    # sliding one-hot: wS[:, 28-4u : 60-4u][c+32g, 4u+g] = 1 for every u
    wS = np.zeros((128, 60), np.float16)
    for g in range(4):
        for c in range(C):
            wS[c + 32 * g, 28 + g] = 1.0

    wNI = (-np.eye(96, dtype=np.float32)).astype(ml_dtypes.bfloat16)

    lnd = np.zeros((96, 8), np.float32)
    for d in range(D):
        j = (D - 1) - d
        for g in range(4):
            lnd[4 * j + g, g] = 1.0
            lnd[4 * j + g, 4 + g] = d
    lnd = lnd.astype(ml_dtypes.bfloat16)

    consts = np.zeros((128, 164), np.uint16)
    consts[:, 0:60] = wS.view(np.uint16)
    consts[0:96, 60:156] = wNI.view(np.uint16)
    consts[0:96, 156:164] = lnd.view(np.uint16)

    # maskcT[4j+g, p'] = MASK_BIAS where (p' mod W) < d (invalid), else 0
    w = np.tile(np.arange(W), QW // W)          # [1600]
    maskcT = np.zeros((96, QW), np.float32)
    for d in range(D):
        j = (D - 1) - d
        row = (w < d).astype(np.float32) * MASK_BIAS
        for g in range(4):
            maskcT[4 * j + g, :] = row
    maskcT = maskcT.astype(ml_dtypes.bfloat16)
    return consts, maskcT


_lock = threading.Lock()
_cache = {}


def _get_program():
    with _lock:
        if "nc" not in _cache:
            _cache["nc"] = _build_program()
            _cache["consts"] = _host_constants()
        return _cache["nc"], _cache["consts"]


def _run(refimg_fea, targetimg_fea, trace=False):
    nc, (consts, maskcT) = _get_program()
    ref = np.ascontiguousarray(refimg_fea, dtype=np.float32)
    tgt = np.ascontiguousarray(targetimg_fea, dtype=np.float32)
    in_maps = []
    for core in range(N_CORES):
        b, hh = core // 2, core % 2
        in_maps.append({
            "ref": ref[b, :, HP * hh:HP * (hh + 1), :].reshape(C, PIX).copy(),
            "tgt": tgt[b, :, HP * hh:HP * (hh + 1), :].reshape(C, PIX).copy(),
            "consts": consts, "maskcT": maskcT,
        })
    res = run_bass_kernel_spmd(nc, in_maps, core_ids=list(range(N_CORES)),
                               trace=trace)
    out = np.empty((B, H, W), np.float32)
    for core in range(N_CORES):
        b, hh = core // 2, core % 2
        o = res.results[core]["out"]           # [8, 1600]: rows g=den, 4+g=num
        pred = (o[4:8] / o[0:4]).reshape(PIX)
        out[b, HP * hh:HP * (hh + 1), :] = pred.reshape(HP, W)
    return out, res


def kernel(refimg_fea, targetimg_fea, maxdisp):
    assert int(maxdisp) == D, f"kernel hardcodes maxdisp={D}, got {maxdisp}"
    out, _ = _run(refimg_fea, targetimg_fea)
    return out


# revision 38
# speedup vs baseline: 1.0053x; 1.0053x over previous
"""HSMNet cost-volume + disparity softmax-regression on 8 Trainium2 NeuronCores.

Reference computation (per batch b):
  cost[c,d,h,w] = |ref[c,h,w] - tgt[c,h,w-d]| for w>=d else 0
  cost_agg[d,h,w] = sum_c cost
  pred[h,w] = sum_d d * softmax_d(cost_agg)

Key identity: |a-b| = 2*max(a,b) - a - b, so
  cost_agg[d,p] = 2*sum_c max(ref[c,p], tgt[c,p-d]) - R[p] - T[p-d]
with R = sum_c ref, T = sum_c tgt. R[p] is constant over d and cancels in the
softmax; the logits used are G[d,p] = 2*S_d[p] - T[p-d]. This removes the
entire elementwise-abs pass; the elementwise work is one tensor_tensor max per
disparity, and several disparities are packed per DVE instruction via an
overlapping-window access pattern (window stride 1 over the pack dim).

Sharding: 8 cores = 4 batches x 2 h-halves (40 rows x 160 cols = 6400 px).
Layout: pixels packed as 4 quarter-groups of 1600 on partitions (c + 32g);
tgt has a 23-col halo so tgt[c, p-d] is a pure column offset.

Per core:
  - 5 batched input DMAs (f32) on the sync queue; casts f32->f16 on GPSIMD
  - DVE: packed tensor_tensor max ops [128, k*800] f16
  - PE: S_d reduced over c by matmuls accumulating into PSUM [96, 2048]
    (row 4j+g, j=23-d, i.e. quadrant q=j//8 + sliding one-hot weights);
    T = sum_c tgt via the same weight tile; T/2 replicated into T_full
    [96,1600] via a DRAM bounce (2 DMAs, 3-dim strided read); -I matmuls
    subtract T_full and the 5000*invalid mask so exp zeroes invalid entries.
  - ACT Exp (scale=2) evacuates PSUM -> E [96,1600] bf16 per 400-col chunk;
    chunks 0-1 pipeline inside the half-2 DVE stream.
  - PE: lnd weights contract E -> den/num [8, 2048] PSUM; DVE evacuates.
  - host: pred = num/den.
"""
import os
import sys
import threading

for _p in ("/opt/trn_rl_repo",):
    if os.path.isdir(_p) and _p not in sys.path:
        sys.path.insert(0, _p)

import numpy as np
import ml_dtypes

import concourse.bacc as bacc
import concourse.mybir as mybir
from concourse.tile import TileContext
from concourse.bass_utils import run_bass_kernel_spmd

dt = mybir.dt

# problem shape (hardcoded per spec)
B, C, H, W = 4, 32, 80, 160
D = 24
HP = H // 2            # rows per core
PIX = HP * W           # 6400 pixels per core
QW = PIX // 4          # 1600 per quarter-group
HW_ = QW // 2          # 800 per half
PAD = 23               # halo columns in front of tgt
N_CORES = 8
MASK_BIAS = 5000.0     # pre-2x logit bias at invalid entries; exp -> 0

# d-pack sizes per 400-col chunk (sum 24); chunk 0 starts small (PE warmup)
PACKS0 = tuple(int(x) for x in os.environ.get("HSM_PACKS0", "2,8,8,6").split(","))
PACKSN = tuple(int(x) for x in os.environ.get("HSM_PACKSN", "8,8,8").split(","))
assert sum(PACKS0) == D and sum(PACKSN) == D
S_BUFS = int(os.environ.get("HSM_S_BUFS", "5"))
CW = 400               # pipeline chunk width (== PSUM bank chunk)


def _win(ap, k):
    """Overlapping-window view: [128, n] slice -> [128, k, n] where window
    j reads columns shifted by +j (stride-1 over the pack dim)."""
    v = ap.unsqueeze(1)
    v.ap[1] = [1, k]
    return v


def _rep(ap, k):
    """Repeat view: [128, n] slice -> [128, k, n], same columns per window."""
    v = ap.unsqueeze(1)
    v.ap[1] = [0, k]
    return v


def _build_program():
    nc = bacc.Bacc("TRN2", target_bir_lowering=False)
    ref_h = nc.dram_tensor("ref", [C, PIX], dt.float32, kind="ExternalInput")
    tgt_h = nc.dram_tensor("tgt", [C, PIX], dt.float32, kind="ExternalInput")
    # consts blob: cols 0:60 wS (f16), 60:156 wNI (bf16), 156:164 lnd (bf16)
    consts_h = nc.dram_tensor("consts", [128, 164], dt.uint16,
                              kind="ExternalInput")
    maskcT_h = nc.dram_tensor("maskcT", [96, QW], dt.bfloat16,
                              kind="ExternalInput")
    # DRAM bounce buffer for the T replication (ExternalOutput so the
    # runtime binds a real buffer; host ignores it)
    dramT = nc.dram_tensor("Tdbg", [4, PAD + QW], dt.bfloat16,
                           kind="ExternalOutput")
    out_h = nc.dram_tensor("out", [8, QW], dt.float32, kind="ExternalOutput")

    Alu = mybir.AluOpType
    Act = mybir.ActivationFunctionType

    # packed [128, x] views of the inputs: partition (g c) <-> hbm (c, 1600g+x)
    def _packedv(th, x0, x1):
        v = th[:, x0:x1].copy()
        v.ap[0] = [QW, 4]
        v.ap.insert(1, [PIX, C])
        return v

    with TileContext(nc) as tc:
        with tc.tile_pool(name="const", bufs=1) as cpool, \
             tc.tile_pool(name="work", bufs=1) as wpool, \
             tc.tile_pool(name="spool", bufs=S_BUFS) as spool:
            consts_sb = cpool.tile([128, 164], dt.uint16)
            wS = consts_sb[:, 0:60].bitcast(dt.float16)
            wNI = consts_sb[0:96, 60:156].bitcast(dt.bfloat16)
            lnd = consts_sb[0:96, 156:164].bitcast(dt.bfloat16)

            refS = [wpool.tile([128, CW], dt.float32, name=f"refS{i}")
                    for i in range(4)]
            tgtS = [wpool.tile([128, PAD + CW], dt.float32, name=f"tgtS{i}")
                    for i in range(4)]
            ref16 = [wpool.tile([128, CW], dt.float16, name=f"r16{i}")
                     for i in range(4)]
            tgt16h = wpool.tile([128, PAD + QW], dt.float16)
            T_hs = wpool.tile([4, PAD + QW], dt.bfloat16)   # T/2 with halo
            T_full = wpool.tile([96, QW], dt.bfloat16)
            maskcT_sb = wpool.tile([96, QW], dt.bfloat16)
            E = [wpool.tile([96, CW], dt.bfloat16, name=f"E{i}")
                 for i in range(4)]
            out_sb = [wpool.tile([8, HW_], dt.float32, name=f"o{i}")
                      for i in range(2)]

            # g=0 halo cols (before pixel 0) are zero
            nc.vector.memset(tgtS[0][0:32, 0:PAD], 0.0)

            # ---- input loads (sync queue), chunk-pipelined ----
            nc.sync.dma_start(consts_sb[:], consts_h[:])
            nc.sync.dma_start(tgtS[0][:, PAD:PAD + CW], _packedv(tgt_h, 0, CW))
            # halo for g>=1: global cols 1600g-23..1600g
            halo_in = tgt_h[:, QW - PAD:QW].copy()
            halo_in.ap[0] = [QW, 3]
            halo_in.ap.insert(1, [PIX, C])
            nc.sync.dma_start(tgtS[0][32:128, 0:PAD], halo_in)
            nc.sync.dma_start(refS[0][:], _packedv(ref_h, 0, CW))
            nc.sync.dma_start(maskcT_sb[:], maskcT_h[:])
            for cc in range(1, 4):
                nc.sync.dma_start(tgtS[cc][:],
                                  _packedv(tgt_h, CW * cc - PAD, CW * cc + CW))
                nc.sync.dma_start(refS[cc][:], _packedv(ref_h, CW * cc, CW * (cc + 1)))

            with tc.tile_pool(name="cost", bufs=1, space="PSUM") as qpool, \
                 tc.tile_pool(name="nd", bufs=1, space="PSUM") as npool:
                cost = [qpool.tile([96, 512], dt.float32, name=f"cost{i}")
                        for i in range(4)]
                nd = npool.tile([8, 2048], dt.float32)
                wT = wS[:, 28:32]  # plain c+32g -> g ones reduction

                started = set()  # (q, cc) PSUM regions already initialized

                def emit_pack(d0, k, cc):
                    # one DVE max op for disparities d0..d0+k-1 (descending
                    # window order), then k channel-reduce matmuls.
                    c0 = CW * cc
                    dhi = d0 + k - 1
                    s = spool.tile([128, k * CW], dt.float16, tag="s",
                                   name=f"s_{cc}_{d0}")
                    base = PAD - dhi + c0
                    nc.vector.tensor_tensor(
                        s[:].rearrange("p (k x) -> p k x", x=CW),
                        _rep(ref16[cc][:], k),
                        _win(tgt16h[:, base:base + CW], k),
                        Alu.max)
                    # window jw of in1 starts at base+jw -> shift PAD-d with
                    # d = dhi-jw (in0 is a broadcast repeat of ref)
                    for jw in range(k):
                        d = dhi - jw
                        j = (D - 1) - d
                        u, q = j % 8, j // 8
                        first = (q, cc) not in started
                        started.add((q, cc))
                        nc.tensor.matmul(
                            cost[cc][32 * q:32 * q + 32, 0:CW],
                            wS[:, 28 - 4 * u:60 - 4 * u],
                            s[:, jw * CW:jw * CW + CW],
                            start=first, stop=False,
                            skip_group_check=True)

                def emit_corr(cc):
                    # T/2 + mask corrections close chunk cc, then exp
                    nc.tensor.matmul(cost[cc][0:96, 0:CW],
                                     wNI, maskcT_sb[:, CW * cc:CW * cc + CW],
                                     start=False, stop=False,
                                     skip_group_check=True)
                    nc.tensor.matmul(cost[cc][0:96, 0:CW],
                                     wNI, T_full[:, CW * cc:CW * cc + CW],
                                     start=False, stop=True,
                                     skip_group_check=True)
                    nc.scalar.activation(E[cc][:], cost[cc][0:96, 0:CW],
                                         Act.Exp, scale=2.0)

                def emit_nd(cc):
                    nc.tensor.matmul(nd[0:8, 512 * cc:512 * cc + CW], lnd,
                                     E[cc][:], start=True, stop=True)

                def emit_evac(i, eng):
                    srcv = nd[0:8, 1024 * i:1024 * i + 1024]
                    srcv = srcv.rearrange("p (k x) -> p k x", k=2)[:, :, 0:CW]
                    dst = out_sb[i][:].rearrange("p (k x) -> p k x", x=CW)
                    eng(dst, srcv)

                # T-red source ranges per chunk and 512-aligned sub-splits
                tred = {0: [(0, 423)], 1: [(423, 512), (512, 823)],
                        2: [(823, 1024), (1024, 1223)],
                        3: [(1223, 1536), (1536, 1623)]}

                # T-red source ranges per chunk and 512-aligned sub-splits
                tred = {0: [(0, 423)], 1: [(423, 512), (512, 823)],
                        2: [(823, 1024), (1024, 1223)],
                        3: [(1223, 1536), (1536, 1623)]}

                for cc in range(4):
                    # casts: tgt on ACT; ref on DVE for chunk 0, else ACT
                    d0c = 0 if cc == 0 else PAD
                    nc.scalar.copy(tgt16h[:, CW * cc + d0c:CW * cc + PAD + CW],
                                   tgtS[cc][:, d0c:PAD + CW])
                    if cc == 0:
                        nc.vector.tensor_copy(ref16[0][:], refS[0][:])
                    else:
                        nc.scalar.copy(ref16[cc][:], refS[cc][:])
                    # T-reduce chunk (PE) into nd temp, halve to bf16 (ACT),
                    # bounce via DRAM (scalar queue DMAs)
                    lo, hi = tred[cc][0][0], tred[cc][-1][1]
                    for a, b in tred[cc]:
                        nc.tensor.matmul(nd[0:4, a:b], wT, tgt16h[:, a:b],
                                         start=True, stop=True)
                    nc.scalar.mul(T_hs[:, lo:hi], nd[0:4, lo:hi], 0.5)
                    nc.sync.dma_start(dramT[:, lo:hi], T_hs[:, lo:hi])
                    tin = dramT[:, CW * cc:CW * cc + CW].copy()
                    tin.ap[0] = [1, D]            # j
                    tin.ap[1] = [PAD + QW, 4]     # g
                    tin.ap.append([1, CW])        # x ; elem = g*1623+j+x+off
                    nc.sync.dma_start(T_full[:, CW * cc:CW * (cc + 1)], tin)
                    # the d-packs + channel reduction for this chunk
                    packs, d0 = [], 0
                    for k in (PACKS0 if cc == 0 else PACKSN):
                        packs.append((d0, k)); d0 += k
                    for p in packs:
                        emit_pack(*p, cc)
                    emit_corr(cc)
                    if cc >= 1:
                        emit_nd(cc - 1)
                    if cc == 3:
                        emit_evac(0, nc.scalar.copy)
                        nc.sync.dma_start(out_h[:, 0:HW_], out_sb[0][:])
                emit_nd(3)
                emit_evac(1, nc.vector.tensor_copy)
                nc.sync.dma_start(out_h[:, HW_:QW], out_sb[1][:])

    nc.compile()
    return nc


def _host_constants():
    # sliding one-hot: wS[:, 28-4u : 60-4u][c+32g, 4u+g] = 1 for every u
    wS = np.zeros((128, 60), np.float16)
    for g in range(4):
        for c in range(C):
            wS[c + 32 * g, 28 + g] = 1.0

    wNI = (-np.eye(96, dtype=np.float32)).astype(ml_dtypes.bfloat16)

    lnd = np.zeros((96, 8), np.float32)
    for d in range(D):
        j = (D - 1) - d
        for g in range(4):
            lnd[4 * j + g, g] = 1.0
            lnd[4 * j + g, 4 + g] = d
    lnd = lnd.astype(ml_dtypes.bfloat16)

    consts = np.zeros((128, 164), np.uint16)
    consts[:, 0:60] = wS.view(np.uint16)
    consts[0:96, 60:156] = wNI.view(np.uint16)
    consts[0:96, 156:164] = lnd.view(np.uint16)

    # maskcT[4j+g, p'] = MASK_BIAS where (p' mod W) < d (invalid), else 0
    w = np.tile(np.arange(W), QW // W)          # [1600]
    maskcT = np.zeros((96, QW), np.float32)
    for d in range(D):
        j = (D - 1) - d
        row = (w < d).astype(np.float32) * MASK_BIAS
        for g in range(4):
            maskcT[4 * j + g, :] = row
    maskcT = maskcT.astype(ml_dtypes.bfloat16)
    return consts, maskcT


_lock = threading.Lock()
_cache = {}


def _get_program():
    with _lock:
        if "nc" not in _cache:
            _cache["nc"] = _build_program()
            _cache["consts"] = _host_constants()
        return _cache["nc"], _cache["consts"]


def _run(refimg_fea, targetimg_fea, trace=False):
    nc, (consts, maskcT) = _get_program()
    ref = np.ascontiguousarray(refimg_fea, dtype=np.float32)
    tgt = np.ascontiguousarray(targetimg_fea, dtype=np.float32)
    in_maps = []
    for core in range(N_CORES):
        b, hh = core // 2, core % 2
        in_maps.append({
            "ref": ref[b, :, HP * hh:HP * (hh + 1), :].reshape(C, PIX).copy(),
            "tgt": tgt[b, :, HP * hh:HP * (hh + 1), :].reshape(C, PIX).copy(),
            "consts": consts, "maskcT": maskcT,
        })
    res = run_bass_kernel_spmd(nc, in_maps, core_ids=list(range(N_CORES)),
                               trace=trace)
    out = np.empty((B, H, W), np.float32)
    for core in range(N_CORES):
        b, hh = core // 2, core % 2
        o = res.results[core]["out"]           # [8, 1600]: rows g=den, 4+g=num
        pred = (o[4:8] / o[0:4]).reshape(PIX)
        out[b, HP * hh:HP * (hh + 1), :] = pred.reshape(HP, W)
    return out, res


def kernel(refimg_fea, targetimg_fea, maxdisp):
    assert int(maxdisp) == D, f"kernel hardcodes maxdisp={D}, got {maxdisp}"
    out, _ = _run(refimg_fea, targetimg_fea)
    return out


# revision 39
# speedup vs baseline: 1.0292x; 1.0238x over previous
"""HSMNet cost-volume + disparity softmax-regression on 8 Trainium2 NeuronCores.

Reference computation (per batch b):
  cost[c,d,h,w] = |ref[c,h,w] - tgt[c,h,w-d]| for w>=d else 0
  cost_agg[d,h,w] = sum_c cost
  pred[h,w] = sum_d d * softmax_d(cost_agg)

Key identity: |a-b| = 2*max(a,b) - a - b, so
  cost_agg[d,p] = 2*sum_c max(ref[c,p], tgt[c,p-d]) - R[p] - T[p-d]
with R = sum_c ref, T = sum_c tgt. R[p] is constant over d and cancels in the
softmax; the logits used are G[d,p] = 2*S_d[p] - T[p-d]. This removes the
entire elementwise-abs pass; the elementwise work is one tensor_tensor max per
disparity, and several disparities are packed per DVE instruction via an
overlapping-window access pattern (window stride 1 over the pack dim).

Sharding: 8 cores = 4 batches x 2 h-halves (40 rows x 160 cols = 6400 px).
Layout: pixels packed as 4 quarter-groups of 1600 on partitions (c + 32g);
tgt has a 23-col halo so tgt[c, p-d] is a pure column offset.

Per core:
  - 5 batched input DMAs (f32) on the sync queue; casts f32->f16 on GPSIMD
  - DVE: packed tensor_tensor max ops [128, k*800] f16
  - PE: S_d reduced over c by matmuls accumulating into PSUM [96, 2048]
    (row 4j+g, j=23-d, i.e. quadrant q=j//8 + sliding one-hot weights);
    T = sum_c tgt via the same weight tile; T/2 replicated into T_full
    [96,1600] via a DRAM bounce (2 DMAs, 3-dim strided read); -I matmuls
    subtract T_full and the 5000*invalid mask so exp zeroes invalid entries.
  - ACT Exp (scale=2) evacuates PSUM -> E [96,1600] bf16 per 400-col chunk;
    chunks 0-1 pipeline inside the half-2 DVE stream.
  - PE: lnd weights contract E -> den/num [8, 2048] PSUM; DVE evacuates.
  - host: pred = num/den.
"""
import os
import sys
import threading

for _p in ("/opt/trn_rl_repo",):
    if os.path.isdir(_p) and _p not in sys.path:
        sys.path.insert(0, _p)

import numpy as np
import ml_dtypes

import concourse.bacc as bacc
import concourse.mybir as mybir
from concourse.tile import TileContext
from concourse.bass_utils import run_bass_kernel_spmd

dt = mybir.dt

# problem shape (hardcoded per spec)
B, C, H, W = 4, 32, 80, 160
D = 24
HP = H // 2            # rows per core
PIX = HP * W           # 6400 pixels per core
QW = PIX // 4          # 1600 per quarter-group
HW_ = QW // 2          # 800 per half
PAD = 23               # halo columns in front of tgt
N_CORES = 8
MASK_BIAS = 5000.0     # pre-2x logit bias at invalid entries; exp -> 0

# d-pack sizes per 400-col chunk (sum 24); chunk 0 starts small (PE warmup)
PACKS0 = tuple(int(x) for x in os.environ.get("HSM_PACKS0", "2,8,8,6").split(","))
PACKSN = tuple(int(x) for x in os.environ.get("HSM_PACKSN", "8,8,8").split(","))
assert sum(PACKS0) == D and sum(PACKSN) == D
S_BUFS = int(os.environ.get("HSM_S_BUFS", "5"))
CW = 400               # pipeline chunk width (== PSUM bank chunk)


def _win(ap, k):
    """Overlapping-window view: [128, n] slice -> [128, k, n] where window
    j reads columns shifted by +j (stride-1 over the pack dim)."""
    v = ap.unsqueeze(1)
    v.ap[1] = [1, k]
    return v


def _rep(ap, k):
    """Repeat view: [128, n] slice -> [128, k, n], same columns per window."""
    v = ap.unsqueeze(1)
    v.ap[1] = [0, k]
    return v


def _build_program():
    nc = bacc.Bacc("TRN2", target_bir_lowering=False)
    ref_h = nc.dram_tensor("ref", [C, PIX], dt.float32, kind="ExternalInput")
    tgt_h = nc.dram_tensor("tgt", [C, PIX], dt.float32, kind="ExternalInput")
    # consts blob: cols 0:60 wS (f16), 60:156 wNI (bf16), 156:164 lnd (bf16)
    consts_h = nc.dram_tensor("consts", [128, 164], dt.uint16,
                              kind="ExternalInput")
    maskcT_h = nc.dram_tensor("maskcT", [96, QW], dt.bfloat16,
                              kind="ExternalInput")
    # DRAM bounce buffer for the T replication (ExternalOutput so the
    # runtime binds a real buffer; host ignores it)
    dramT = nc.dram_tensor("Tdbg", [4, PAD + QW], dt.bfloat16,
                           kind="ExternalOutput")
    out_h = nc.dram_tensor("out", [8, QW], dt.float32, kind="ExternalOutput")

    Alu = mybir.AluOpType
    Act = mybir.ActivationFunctionType

    # packed [128, x] views of the inputs: partition (g c) <-> hbm (c, 1600g+x)
    def _packedv(th, x0, x1):
        v = th[:, x0:x1].copy()
        v.ap[0] = [QW, 4]
        v.ap.insert(1, [PIX, C])
        return v

    with TileContext(nc) as tc:
        with tc.tile_pool(name="const", bufs=1) as cpool, \
             tc.tile_pool(name="work", bufs=1) as wpool, \
             tc.tile_pool(name="spool", bufs=S_BUFS) as spool:
            consts_sb = cpool.tile([128, 164], dt.uint16)
            wS = consts_sb[:, 0:60].bitcast(dt.float16)
            wNI = consts_sb[0:96, 60:156].bitcast(dt.bfloat16)
            lnd = consts_sb[0:96, 156:164].bitcast(dt.bfloat16)

            refS = [wpool.tile([128, CW], dt.float32, name=f"refS{i}")
                    for i in range(4)]
            tgtS = [wpool.tile([128, PAD + CW], dt.float32, name=f"tgtS{i}")
                    for i in range(4)]
            ref16 = [wpool.tile([128, CW], dt.float16, name=f"r16{i}")
                     for i in range(4)]
            tgt16h = wpool.tile([128, PAD + QW], dt.float16)
            T_hs = wpool.tile([4, PAD + QW], dt.bfloat16)   # T/2 with halo
            T_full = wpool.tile([96, QW], dt.bfloat16)
            maskcT_sb = wpool.tile([96, QW], dt.bfloat16)
            E = [wpool.tile([96, CW], dt.bfloat16, name=f"E{i}")
                 for i in range(4)]
            out_sb = [wpool.tile([8, HW_], dt.float32, name=f"o{i}")
                      for i in range(2)]

            # g=0 halo cols (before pixel 0) are zero
            nc.vector.memset(tgtS[0][0:32, 0:PAD], 0.0)

            # ---- input loads (sync queue), chunk-pipelined ----
            nc.sync.dma_start(consts_sb[:], consts_h[:])
            nc.sync.dma_start(tgtS[0][:, PAD:PAD + CW], _packedv(tgt_h, 0, CW))
            # halo for g>=1: global cols 1600g-23..1600g
            halo_in = tgt_h[:, QW - PAD:QW].copy()
            halo_in.ap[0] = [QW, 3]
            halo_in.ap.insert(1, [PIX, C])
            nc.sync.dma_start(tgtS[0][32:128, 0:PAD], halo_in)
            nc.sync.dma_start(refS[0][:], _packedv(ref_h, 0, CW))
            nc.sync.dma_start(maskcT_sb[:], maskcT_h[:])
            for cc in range(1, 4):
                nc.sync.dma_start(tgtS[cc][:],
                                  _packedv(tgt_h, CW * cc - PAD, CW * cc + CW))
                nc.sync.dma_start(refS[cc][:], _packedv(ref_h, CW * cc, CW * (cc + 1)))

            with tc.tile_pool(name="cost", bufs=1, space="PSUM") as qpool, \
                 tc.tile_pool(name="nd", bufs=1, space="PSUM") as npool:
                cost = [qpool.tile([96, 512], dt.float32, name=f"cost{i}")
                        for i in range(4)]
                nd = npool.tile([8, 2048], dt.float32)
                wT = wS[:, 28:32]  # plain c+32g -> g ones reduction

                started = set()  # (q, cc) PSUM regions already initialized

                def emit_pack(d0, k, cc):
                    # one DVE max op for disparities d0..d0+k-1 (descending
                    # window order), then k channel-reduce matmuls.
                    c0 = CW * cc
                    dhi = d0 + k - 1
                    s = spool.tile([128, k * CW], dt.float16, tag="s",
                                   name=f"s_{cc}_{d0}")
                    base = PAD - dhi + c0
                    nc.vector.tensor_tensor(
                        s[:].rearrange("p (k x) -> p k x", x=CW),
                        _rep(ref16[cc][:], k),
                        _win(tgt16h[:, base:base + CW], k),
                        Alu.max)
                    # window jw of in1 starts at base+jw -> shift PAD-d with
                    # d = dhi-jw (in0 is a broadcast repeat of ref)
                    for jw in range(k):
                        d = dhi - jw
                        j = (D - 1) - d
                        u, q = j % 8, j // 8
                        first = (q, cc) not in started
                        started.add((q, cc))
                        nc.tensor.matmul(
                            cost[cc][32 * q:32 * q + 32, 0:CW],
                            wS[:, 28 - 4 * u:60 - 4 * u],
                            s[:, jw * CW:jw * CW + CW],
                            start=first, stop=False,
                            skip_group_check=True)

                def emit_corr(cc):
                    # T/2 + mask corrections close chunk cc, then exp
                    nc.tensor.matmul(cost[cc][0:96, 0:CW],
                                     wNI, maskcT_sb[:, CW * cc:CW * cc + CW],
                                     start=False, stop=False,
                                     skip_group_check=True)
                    nc.tensor.matmul(cost[cc][0:96, 0:CW],
                                     wNI, T_full[:, CW * cc:CW * cc + CW],
                                     start=False, stop=True,
                                     skip_group_check=True)
                    nc.scalar.activation(E[cc][:], cost[cc][0:96, 0:CW],
                                         Act.Exp, scale=2.0)

                def emit_nd(cc):
                    nc.tensor.matmul(nd[0:8, 512 * cc:512 * cc + CW], lnd,
                                     E[cc][:], start=True, stop=True)

                def emit_evac(i, eng):
                    srcv = nd[0:8, 1024 * i:1024 * i + 1024]
                    srcv = srcv.rearrange("p (k x) -> p k x", k=2)[:, :, 0:CW]
                    dst = out_sb[i][:].rearrange("p (k x) -> p k x", x=CW)
                    eng(dst, srcv)

                # T-red source ranges per chunk and 512-aligned sub-splits
                tred = {0: [(0, 423)], 1: [(423, 512), (512, 823)],
                        2: [(823, 1024), (1024, 1223)],
                        3: [(1223, 1536), (1536, 1623)]}

                # T-red source ranges per chunk and 512-aligned sub-splits
                tred = {0: [(0, 423)], 1: [(423, 512), (512, 823)],
                        2: [(823, 1024), (1024, 1223)],
                        3: [(1223, 1536), (1536, 1623)]}

                for cc in range(4):
                    # casts: tgt on ACT; ref on DVE for chunk 0, else ACT
                    d0c = 0 if cc == 0 else PAD
                    nc.scalar.copy(tgt16h[:, CW * cc + d0c:CW * cc + PAD + CW],
                                   tgtS[cc][:, d0c:PAD + CW])
                    if cc == 0:
                        nc.vector.tensor_copy(ref16[0][:], refS[0][:])
                    else:
                        nc.scalar.copy(ref16[cc][:], refS[cc][:])
                    # T-reduce chunk (PE) into nd temp, halve to bf16 (ACT),
                    # bounce via DRAM (scalar queue DMAs)
                    lo, hi = tred[cc][0][0], tred[cc][-1][1]
                    for a, b in tred[cc]:
                        nc.tensor.matmul(nd[0:4, a:b], wT, tgt16h[:, a:b],
                                         start=True, stop=True)
                    nc.scalar.mul(T_hs[:, lo:hi], nd[0:4, lo:hi], 0.5)
                    nc.scalar.dma_start(dramT[:, lo:hi], T_hs[:, lo:hi])
                    tin = dramT[:, CW * cc:CW * cc + CW].copy()
                    tin.ap[0] = [1, D]            # j
                    tin.ap[1] = [PAD + QW, 4]     # g
                    tin.ap.append([1, CW])        # x ; elem = g*1623+j+x+off
                    nc.scalar.dma_start(T_full[:, CW * cc:CW * (cc + 1)], tin)
                    # the d-packs + channel reduction for this chunk
                    packs, d0 = [], 0
                    for k in (PACKS0 if cc == 0 else PACKSN):
                        packs.append((d0, k)); d0 += k
                    for p in packs:
                        emit_pack(*p, cc)
                    emit_corr(cc)
                    if cc >= 1:
                        emit_nd(cc - 1)
                    if cc == 3:
                        emit_evac(0, nc.scalar.copy)
                        nc.sync.dma_start(out_h[:, 0:HW_], out_sb[0][:])
                emit_nd(3)
                emit_evac(1, nc.vector.tensor_copy)
                nc.sync.dma_start(out_h[:, HW_:QW], out_sb[1][:])

    nc.compile()
    return nc


def _host_constants():
    # sliding one-hot: wS[:, 28-4u : 60-4u][c+32g, 4u+g] = 1 for every u
    wS = np.zeros((128, 60), np.float16)
    for g in range(4):
        for c in range(C):
            wS[c + 32 * g, 28 + g] = 1.0

    wNI = (-np.eye(96, dtype=np.float32)).astype(ml_dtypes.bfloat16)

    lnd = np.zeros((96, 8), np.float32)
    for d in range(D):
        j = (D - 1) - d
        for g in range(4):
            lnd[4 * j + g, g] = 1.0
            lnd[4 * j + g, 4 + g] = d
    lnd = lnd.astype(ml_dtypes.bfloat16)

    consts = np.zeros((128, 164), np.uint16)
    consts[:, 0:60] = wS.view(np.uint16)
    consts[0:96, 60:156] = wNI.view(np.uint16)
    consts[0:96, 156:164] = lnd.view(np.uint16)

    # maskcT[4j+g, p'] = MASK_BIAS where (p' mod W) < d (invalid), else 0
    w = np.tile(np.arange(W), QW // W)          # [1600]
    maskcT = np.zeros((96, QW), np.float32)
    for d in range(D):
        j = (D - 1) - d
        row = (w < d).astype(np.float32) * MASK_BIAS
        for g in range(4):
            maskcT[4 * j + g, :] = row
    maskcT = maskcT.astype(ml_dtypes.bfloat16)
    return consts, maskcT


_lock = threading.Lock()
_cache = {}


def _get_program():
    with _lock:
        if "nc" not in _cache:
            _cache["nc"] = _build_program()
            _cache["consts"] = _host_constants()
        return _cache["nc"], _cache["consts"]


def _run(refimg_fea, targetimg_fea, trace=False):
    nc, (consts, maskcT) = _get_program()
    ref = np.ascontiguousarray(refimg_fea, dtype=np.float32)
    tgt = np.ascontiguousarray(targetimg_fea, dtype=np.float32)
    in_maps = []
    for core in range(N_CORES):
        b, hh = core // 2, core % 2
        in_maps.append({
            "ref": ref[b, :, HP * hh:HP * (hh + 1), :].reshape(C, PIX).copy(),
            "tgt": tgt[b, :, HP * hh:HP * (hh + 1), :].reshape(C, PIX).copy(),
            "consts": consts, "maskcT": maskcT,
        })
    res = run_bass_kernel_spmd(nc, in_maps, core_ids=list(range(N_CORES)),
                               trace=trace)
    out = np.empty((B, H, W), np.float32)
    for core in range(N_CORES):
        b, hh = core // 2, core % 2
        o = res.results[core]["out"]           # [8, 1600]: rows g=den, 4+g=num
        pred = (o[4:8] / o[0:4]).reshape(PIX)
        out[b, HP * hh:HP * (hh + 1), :] = pred.reshape(HP, W)
    return out, res


def kernel(refimg_fea, targetimg_fea, maxdisp):
    assert int(maxdisp) == D, f"kernel hardcodes maxdisp={D}, got {maxdisp}"
    out, _ = _run(refimg_fea, targetimg_fea)
    return out


# revision 40
# speedup vs baseline: 1.0394x; 1.0099x over previous
"""HSMNet cost-volume + disparity softmax-regression on 8 Trainium2 NeuronCores.

Reference computation (per batch b):
  cost[c,d,h,w] = |ref[c,h,w] - tgt[c,h,w-d]| for w>=d else 0
  cost_agg[d,h,w] = sum_c cost
  pred[h,w] = sum_d d * softmax_d(cost_agg)

Key identity: |a-b| = 2*max(a,b) - a - b, so
  cost_agg[d,p] = 2*sum_c max(ref[c,p], tgt[c,p-d]) - R[p] - T[p-d]
with R = sum_c ref, T = sum_c tgt. R[p] is constant over d and cancels in the
softmax; the logits used are G[d,p] = 2*S_d[p] - T[p-d]. This removes the
entire elementwise-abs pass; the elementwise work is one tensor_tensor max per
disparity, and several disparities are packed per DVE instruction via an
overlapping-window access pattern (window stride 1 over the pack dim).

Sharding: 8 cores = 4 batches x 2 h-halves (40 rows x 160 cols = 6400 px).
Layout: pixels packed as 4 quarter-groups of 1600 on partitions (c + 32g);
tgt has a 23-col halo so tgt[c, p-d] is a pure column offset.

Per core:
  - 5 batched input DMAs (f32) on the sync queue; casts f32->f16 on GPSIMD
  - DVE: packed tensor_tensor max ops [128, k*800] f16
  - PE: S_d reduced over c by matmuls accumulating into PSUM [96, 2048]
    (row 4j+g, j=23-d, i.e. quadrant q=j//8 + sliding one-hot weights);
    T = sum_c tgt via the same weight tile; T/2 replicated into T_full
    [96,1600] via a DRAM bounce (2 DMAs, 3-dim strided read); -I matmuls
    subtract T_full and the 5000*invalid mask so exp zeroes invalid entries.
  - ACT Exp (scale=2) evacuates PSUM -> E [96,1600] bf16 per 400-col chunk;
    chunks 0-1 pipeline inside the half-2 DVE stream.
  - PE: lnd weights contract E -> den/num [8, 2048] PSUM; DVE evacuates.
  - host: pred = num/den.
"""
import os
import sys
import threading

for _p in ("/opt/trn_rl_repo",):
    if os.path.isdir(_p) and _p not in sys.path:
        sys.path.insert(0, _p)

import numpy as np
import ml_dtypes

import concourse.bacc as bacc
import concourse.mybir as mybir
from concourse.tile import TileContext
from concourse.bass_utils import run_bass_kernel_spmd

dt = mybir.dt

# problem shape (hardcoded per spec)
B, C, H, W = 4, 32, 80, 160
D = 24
HP = H // 2            # rows per core
PIX = HP * W           # 6400 pixels per core
QW = PIX // 4          # 1600 per quarter-group
HW_ = QW // 2          # 800 per half
PAD = 23               # halo columns in front of tgt
N_CORES = 8
MASK_BIAS = 5000.0     # pre-2x logit bias at invalid entries; exp -> 0

# d-pack sizes per 400-col chunk (sum 24); chunk 0 starts small (PE warmup)
PACKS0 = tuple(int(x) for x in os.environ.get("HSM_PACKS0", "2,8,8,6").split(","))
PACKSN = tuple(int(x) for x in os.environ.get("HSM_PACKSN", "8,8,8").split(","))
assert sum(PACKS0) == D and sum(PACKSN) == D
S_BUFS = int(os.environ.get("HSM_S_BUFS", "5"))
CW = 400               # pipeline chunk width (== PSUM bank chunk)


def _win(ap, k):
    """Overlapping-window view: [128, n] slice -> [128, k, n] where window
    j reads columns shifted by +j (stride-1 over the pack dim)."""
    v = ap.unsqueeze(1)
    v.ap[1] = [1, k]
    return v


def _rep(ap, k):
    """Repeat view: [128, n] slice -> [128, k, n], same columns per window."""
    v = ap.unsqueeze(1)
    v.ap[1] = [0, k]
    return v


def _build_program():
    nc = bacc.Bacc("TRN2", target_bir_lowering=False)
    ref_h = nc.dram_tensor("ref", [C, PIX], dt.float32, kind="ExternalInput")
    tgt_h = nc.dram_tensor("tgt", [C, PIX], dt.float32, kind="ExternalInput")
    # consts blob: cols 0:60 wS (f16), 60:156 wNI (bf16), 156:164 lnd (bf16)
    consts_h = nc.dram_tensor("consts", [128, 164], dt.uint16,
                              kind="ExternalInput")
    maskcT_h = nc.dram_tensor("maskcT", [96, QW], dt.bfloat16,
                              kind="ExternalInput")
    # DRAM bounce buffer for the T replication (ExternalOutput so the
    # runtime binds a real buffer; host ignores it)
    dramT = nc.dram_tensor("Tdbg", [4, PAD + QW], dt.bfloat16,
                           kind="ExternalOutput")
    out_h = nc.dram_tensor("out", [8, QW], dt.float32, kind="ExternalOutput")

    Alu = mybir.AluOpType
    Act = mybir.ActivationFunctionType

    # packed [128, x] views of the inputs: partition (g c) <-> hbm (c, 1600g+x)
    def _packedv(th, x0, x1):
        v = th[:, x0:x1].copy()
        v.ap[0] = [QW, 4]
        v.ap.insert(1, [PIX, C])
        return v

    with TileContext(nc) as tc:
        with tc.tile_pool(name="const", bufs=1) as cpool, \
             tc.tile_pool(name="work", bufs=1) as wpool, \
             tc.tile_pool(name="spool", bufs=S_BUFS) as spool:
            consts_sb = cpool.tile([128, 164], dt.uint16)
            wS = consts_sb[:, 0:60].bitcast(dt.float16)
            wNI = consts_sb[0:96, 60:156].bitcast(dt.bfloat16)
            lnd = consts_sb[0:96, 156:164].bitcast(dt.bfloat16)

            refS = [wpool.tile([128, CW], dt.float32, name=f"refS{i}")
                    for i in range(4)]
            tgtS = [wpool.tile([128, PAD + CW], dt.float32, name=f"tgtS{i}")
                    for i in range(4)]
            ref16 = [wpool.tile([128, CW], dt.float16, name=f"r16{i}")
                     for i in range(4)]
            tgt16h = wpool.tile([128, PAD + QW], dt.float16)
            T_hs = wpool.tile([4, PAD + QW], dt.bfloat16)   # T/2 with halo
            T_full = wpool.tile([96, QW], dt.bfloat16)
            maskcT_sb = wpool.tile([96, QW], dt.bfloat16)
            E = [wpool.tile([96, CW], dt.bfloat16, name=f"E{i}")
                 for i in range(4)]
            out_sb = [wpool.tile([8, HW_], dt.float32, name=f"o{i}")
                      for i in range(2)]

            # g=0 halo cols (before pixel 0) are zero
            nc.vector.memset(tgtS[0][0:32, 0:PAD], 0.0)

            # ---- input loads (sync queue), chunk-pipelined ----
            nc.sync.dma_start(consts_sb[:], consts_h[:])
            nc.sync.dma_start(tgtS[0][:, PAD:PAD + CW], _packedv(tgt_h, 0, CW))
            # halo for g>=1: global cols 1600g-23..1600g
            halo_in = tgt_h[:, QW - PAD:QW].copy()
            halo_in.ap[0] = [QW, 3]
            halo_in.ap.insert(1, [PIX, C])
            nc.sync.dma_start(tgtS[0][32:128, 0:PAD], halo_in)
            nc.sync.dma_start(refS[0][:], _packedv(ref_h, 0, CW))
            nc.sync.dma_start(maskcT_sb[:], maskcT_h[:])
            for cc in range(1, 4):
                nc.sync.dma_start(tgtS[cc][:],
                                  _packedv(tgt_h, CW * cc - PAD, CW * cc + CW))
                nc.sync.dma_start(refS[cc][:], _packedv(ref_h, CW * cc, CW * (cc + 1)))

            with tc.tile_pool(name="cost", bufs=1, space="PSUM") as qpool, \
                 tc.tile_pool(name="nd", bufs=1, space="PSUM") as npool:
                cost = [qpool.tile([96, 512], dt.float32, name=f"cost{i}")
                        for i in range(4)]
                nd = npool.tile([8, 2048], dt.float32)
                wT = wS[:, 28:32]  # plain c+32g -> g ones reduction

                started = set()  # (q, cc) PSUM regions already initialized

                def emit_pack(d0, k, cc):
                    # one DVE max op for disparities d0..d0+k-1 (descending
                    # window order), then k channel-reduce matmuls.
                    c0 = CW * cc
                    dhi = d0 + k - 1
                    s = spool.tile([128, k * CW], dt.float16, tag="s",
                                   name=f"s_{cc}_{d0}")
                    base = PAD - dhi + c0
                    nc.vector.tensor_tensor(
                        s[:].rearrange("p (k x) -> p k x", x=CW),
                        _rep(ref16[cc][:], k),
                        _win(tgt16h[:, base:base + CW], k),
                        Alu.max)
                    # window jw of in1 starts at base+jw -> shift PAD-d with
                    # d = dhi-jw (in0 is a broadcast repeat of ref)
                    for jw in range(k):
                        d = dhi - jw
                        j = (D - 1) - d
                        u, q = j % 8, j // 8
                        first = (q, cc) not in started
                        started.add((q, cc))
                        nc.tensor.matmul(
                            cost[cc][32 * q:32 * q + 32, 0:CW],
                            wS[:, 28 - 4 * u:60 - 4 * u],
                            s[:, jw * CW:jw * CW + CW],
                            start=first, stop=False,
                            skip_group_check=True)

                def emit_corr(cc):
                    # T/2 + mask corrections close chunk cc, then exp
                    nc.tensor.matmul(cost[cc][0:96, 0:CW],
                                     wNI, maskcT_sb[:, CW * cc:CW * cc + CW],
                                     start=False, stop=False,
                                     skip_group_check=True)
                    nc.tensor.matmul(cost[cc][0:96, 0:CW],
                                     wNI, T_full[:, CW * cc:CW * cc + CW],
                                     start=False, stop=True,
                                     skip_group_check=True)
                    nc.scalar.activation(E[cc][:], cost[cc][0:96, 0:CW],
                                         Act.Exp, scale=2.0)

                def emit_nd(cc):
                    nc.tensor.matmul(nd[0:8, 512 * cc:512 * cc + CW], lnd,
                                     E[cc][:], start=True, stop=True)

                def emit_evac(i, eng):
                    srcv = nd[0:8, 1024 * i:1024 * i + 1024]
                    srcv = srcv.rearrange("p (k x) -> p k x", k=2)[:, :, 0:CW]
                    dst = out_sb[i][:].rearrange("p (k x) -> p k x", x=CW)
                    eng(dst, srcv)

                # T-red source ranges per chunk and 512-aligned sub-splits
                tred = {0: [(0, 423)], 1: [(423, 512), (512, 823)],
                        2: [(823, 1024), (1024, 1223)],
                        3: [(1223, 1536), (1536, 1623)]}

                # T-red source ranges per chunk and 512-aligned sub-splits
                tred = {0: [(0, 423)], 1: [(423, 512), (512, 823)],
                        2: [(823, 1024), (1024, 1223)],
                        3: [(1223, 1536), (1536, 1623)]}

                def emit_casts(cc):
                    # tgt on ACT; ref on DVE for chunk 0, else ACT
                    d0c = 0 if cc == 0 else PAD
                    nc.scalar.copy(tgt16h[:, CW * cc + d0c:CW * cc + PAD + CW],
                                   tgtS[cc][:, d0c:PAD + CW])
                    if cc == 0:
                        nc.vector.tensor_copy(ref16[0][:], refS[0][:])
                    else:
                        nc.scalar.copy(ref16[cc][:], refS[cc][:])

                casted = {0}
                emit_casts(0)
                for cc in range(4):
                    # T-reduce chunk (PE) into nd temp, halve to bf16 (ACT),
                    # bounce via DRAM (scalar queue DMAs)
                    lo, hi = tred[cc][0][0], tred[cc][-1][1]
                    for a, b in tred[cc]:
                        nc.tensor.matmul(nd[0:4, a:b], wT, tgt16h[:, a:b],
                                         start=True, stop=True)
                    nc.scalar.mul(T_hs[:, lo:hi], nd[0:4, lo:hi], 0.5)
                    nc.scalar.dma_start(dramT[:, lo:hi], T_hs[:, lo:hi])
                    tin = dramT[:, CW * cc:CW * cc + CW].copy()
                    tin.ap[0] = [1, D]            # j
                    tin.ap[1] = [PAD + QW, 4]     # g
                    tin.ap.append([1, CW])        # x ; elem = g*1623+j+x+off
                    nc.scalar.dma_start(T_full[:, CW * cc:CW * (cc + 1)], tin)
                    # the d-packs + channel reduction for this chunk
                    packs, d0 = [], 0
                    for k in (PACKS0 if cc == 0 else PACKSN):
                        packs.append((d0, k)); d0 += k
                    for p in packs:
                        emit_pack(*p, cc)
                    # hoist upcoming chunks' casts ahead of this chunk's exp
                    # (exp waits on the whole chunk and would block them)
                    for nx in (cc + 1, cc + 2):
                        if nx <= 3 and nx not in casted:
                            casted.add(nx)
                            emit_casts(nx)
                    emit_corr(cc)
                    if cc >= 1:
                        emit_nd(cc - 1)
                    if cc == 3:
                        emit_evac(0, nc.scalar.copy)
                        nc.sync.dma_start(out_h[:, 0:HW_], out_sb[0][:])
                emit_nd(3)
                emit_evac(1, nc.vector.tensor_copy)
                nc.sync.dma_start(out_h[:, HW_:QW], out_sb[1][:])

    nc.compile()
    return nc


def _host_constants():
    # sliding one-hot: wS[:, 28-4u : 60-4u][c+32g, 4u+g] = 1 for every u
    wS = np.zeros((128, 60), np.float16)
    for g in range(4):
        for c in range(C):
            wS[c + 32 * g, 28 + g] = 1.0

    wNI = (-np.eye(96, dtype=np.float32)).astype(ml_dtypes.bfloat16)

    lnd = np.zeros((96, 8), np.float32)
    for d in range(D):
        j = (D - 1) - d
        for g in range(4):
            lnd[4 * j + g, g] = 1.0
            lnd[4 * j + g, 4 + g] = d
    lnd = lnd.astype(ml_dtypes.bfloat16)

    consts = np.zeros((128, 164), np.uint16)
    consts[:, 0:60] = wS.view(np.uint16)
    consts[0:96, 60:156] = wNI.view(np.uint16)
    consts[0:96, 156:164] = lnd.view(np.uint16)

    # maskcT[4j+g, p'] = MASK_BIAS where (p' mod W) < d (invalid), else 0
    w = np.tile(np.arange(W), QW // W)          # [1600]
    maskcT = np.zeros((96, QW), np.float32)
    for d in range(D):
        j = (D - 1) - d
        row = (w < d).astype(np.float32) * MASK_BIAS
        for g in range(4):
            maskcT[4 * j + g, :] = row
    maskcT = maskcT.astype(ml_dtypes.bfloat16)
    return consts, maskcT


_lock = threading.Lock()
_cache = {}


def _get_program():
    with _lock:
        if "nc" not in _cache:
            _cache["nc"] = _build_program()
            _cache["consts"] = _host_constants()
        return _cache["nc"], _cache["consts"]


def _run(refimg_fea, targetimg_fea, trace=False):
    nc, (consts, maskcT) = _get_program()
    ref = np.ascontiguousarray(refimg_fea, dtype=np.float32)
    tgt = np.ascontiguousarray(targetimg_fea, dtype=np.float32)
    in_maps = []
    for core in range(N_CORES):
        b, hh = core // 2, core % 2
        in_maps.append({
            "ref": ref[b, :, HP * hh:HP * (hh + 1), :].reshape(C, PIX).copy(),
            "tgt": tgt[b, :, HP * hh:HP * (hh + 1), :].reshape(C, PIX).copy(),
            "consts": consts, "maskcT": maskcT,
        })
    res = run_bass_kernel_spmd(nc, in_maps, core_ids=list(range(N_CORES)),
                               trace=trace)
    out = np.empty((B, H, W), np.float32)
    for core in range(N_CORES):
        b, hh = core // 2, core % 2
        o = res.results[core]["out"]           # [8, 1600]: rows g=den, 4+g=num
        pred = (o[4:8] / o[0:4]).reshape(PIX)
        out[b, HP * hh:HP * (hh + 1), :] = pred.reshape(HP, W)
    return out, res


def kernel(refimg_fea, targetimg_fea, maxdisp):
    assert int(maxdisp) == D, f"kernel hardcodes maxdisp={D}, got {maxdisp}"
    out, _ = _run(refimg_fea, targetimg_fea)
    return out
